# revision 1
# baseline (speedup 1.0000x reference)
"""DGACritic forward as a Bass/Tile kernel on 8 trn2 NeuronCores.

Data-parallel over batch. Per core: feature-major layout built by PE
matmul-transposes; algebraic fusions: q/k projections folded into one
bilinear matrix per group (logits_m = p.T tok_m with p = A.T tok_0),
v-projection eliminated (h = avW.T (sum_m w_m tok_m)), softmax
normalization deferred past the value matmul.

Host path is copy-free: states/actions are fed to the device as-is
(f32, batch-sharded row slices via device_put — no interleave, no
concat, no dtype cast on host). The 48+16 per-agent feature interleave
happens on-chip with strided engine copies, and the output is
reordered on-chip so y comes back in natural batch order.

Batch within a super-tile is processed in a permuted order
b = p*NBC + c  ->  sbuf free position c*128 + p, so that input DMAs
read one contiguous run per partition; an on-chip reorder before the
output DMA inverts the permutation.
"""

import math
import sys

sys.path.insert(0, "/opt/trn_rl_repo")

import numpy as np
import ml_dtypes

import concourse.bass as bass
import concourse.bacc as bacc
import concourse.mybir as mybir
from concourse.tile import TileContext
from concourse import bass_utils

BF16 = ml_dtypes.bfloat16
F32 = mybir.dt.float32
BT16 = mybir.dt.bfloat16

N_CORES = 8
B_FULL = 131072
NA, S, A, D, H = 8, 48, 16, 128, 256
FS, FA = NA * S, NA * A  # 384, 128
SCALE = 1.0 / math.sqrt(D)
BPC = B_FULL // N_CORES  # 16384
ST = 2048                # batch super-tile (free dim for elementwise)
NBC = ST // 128          # 16 batch chunks per super-tile
NSUB = ST // 512         # matmul N=512 subtiles per super-tile

AX = mybir.AluOpType
AF = mybir.ActivationFunctionType


def _emit(nc, bpc, reps=1, stage=99):
    nst = bpc // ST
    f32, bf = F32, BT16

    xs = nc.dram_tensor("xs", [bpc, FS], f32, kind="ExternalInput").ap()
    xa = nc.dram_tensor("xa", [bpc, FA], f32, kind="ExternalInput").ap()
    ident_d = nc.dram_tensor("ident", [128, 128], bf, kind="ExternalInput").ap()
    wtok_d = nc.dram_tensor("wtok", [128, 512], bf, kind="ExternalInput").ap()
    wattn_d = nc.dram_tensor("wattn", [128, 512], bf, kind="ExternalInput").ap()
    wgate_d = nc.dram_tensor("wgate", [128, 256], bf, kind="ExternalInput").ap()
    w1sa_d = nc.dram_tensor("w1sa", [64, 256], bf, kind="ExternalInput").ap()
    w1E_d = nc.dram_tensor("w1E", [128, 256], bf, kind="ExternalInput").ap()
    w2_d = nc.dram_tensor("w2", [128, 512], bf, kind="ExternalInput").ap()
    w3_d = nc.dram_tensor("w3", [128, 2], bf, kind="ExternalInput").ap()
    ones_d = nc.dram_tensor("ones", [128, 128], bf, kind="ExternalInput").ap()
    bias_d = nc.dram_tensor("biasm", [128, 16], f32, kind="ExternalInput").ap()
    y = nc.dram_tensor("y", [bpc, 1], f32, kind="ExternalOutput").ap()

    act, dve, gps, pe = nc.scalar, nc.vector, nc.gpsimd, nc.tensor

    from contextlib import ExitStack

    with TileContext(nc) as tc, ExitStack() as es:
        wp = es.enter_context(tc.tile_pool(name="wp", bufs=1))
        iop = es.enter_context(tc.tile_pool(name="iop", bufs=2))
        xbp = es.enter_context(tc.tile_pool(name="xbp", bufs=2))
        xtp = es.enter_context(tc.tile_pool(name="xtp", bufs=2))
        tkp = es.enter_context(tc.tile_pool(name="tkp", bufs=8))
        ep = es.enter_context(tc.tile_pool(name="ep", bufs=3))
        up = es.enter_context(tc.tile_pool(name="up", bufs=2))
        mid = es.enter_context(tc.tile_pool(name="mid", bufs=2))
        one = es.enter_context(tc.tile_pool(name="one", bufs=2))
        # one shared PSUM ring: 4 tiles x [128, 1024] f32 = all 8 banks
        psp = es.enter_context(tc.tile_pool(name="psp", bufs=4, space="PSUM"))

        # ---- load constants/weights into SBUF once ----
        def wload(name, shape, dt, src):
            t = wp.tile(shape, dt, tag=name)
            nc.sync.dma_start(t, src)
            return t

        ident = wload("ident", [128, 128], bf, ident_d)
        wtok = wload("wtok", [128, 512], bf, wtok_d)
        wattn = wload("wattn", [128, 512], bf, wattn_d)
        wgate = wload("wgate", [128, 256], bf, wgate_d)
        w1sa = wload("w1sa", [64, 256], bf, w1sa_d)
        w1E = wload("w1E", [128, 256], bf, w1E_d)
        w2 = wload("w2", [128, 512], bf, w2_d)
        w3 = wload("w3", [128, 2], bf, w3_d)
        ones = wload("ones", [128, 128], bf, ones_d)
        bm = wload("biasm", [128, 16], f32, bias_d)

        def bcol(i):  # per-partition bias column AP
            return bm[:, i : i + 1]

        xs_v = xs.rearrange("(q p c) f -> q p c f", p=128, c=NBC)
        xa_v = xa.rearrange("(q p c) f -> q p c f", p=128, c=NBC)

        HB = 1024               # PSUM tile width (2 banks)
        NH = ST // HB           # halves per super-tile

        def hsl(h):
            return slice(h * HB, (h + 1) * HB)

        dummy = None
        if stage < 99:
            dummy = one.tile([1, ST], f32, tag="dummy", bufs=1)
            nc.vector.memset(dummy, 0.0)

        for st in range(nst * reps):
            st = st % nst
            # ---------- phase T: load + interleave + transpose ----------
            # xT layout: [fpair(128 partitions), fc(4), ST] bf16; free pos c*128+p
            xT = xtp.tile([128, 4, ST], bf, tag="xT")
            for qh in range(4):  # quarters of the super-tile: c in [qh*4, qh*4+4)
                cs = slice(qh * 4, qh * 4 + 4)
                xsb = iop.tile([128, 4, FS], f32, tag="xsb")
                xab = iop.tile([128, 4, FA], f32, tag="xab")
                nc.sync.dma_start(xsb, xs_v[st, :, cs, :])
                nc.sync.dma_start(xab, xa_v[st, :, cs, :])
                # interleave to per-agent [48 s | 16 a] blocks, f32 -> bf16
                if stage < 1:
                    continue
                xb = xbp.tile([128, 4, 512], bf, tag="xb")
                xb8 = xb.rearrange("p i (n f) -> p i n f", n=8)
                gps.tensor_copy(xb8[:, :, :, 0:S],
                                xsb.rearrange("p i (n f) -> p i n f", n=8))
                gps.tensor_copy(xb8[:, :, :, S:64],
                                xab.rearrange("p i (n f) -> p i n f", n=8))
                if stage < 2:
                    continue
                if stage == 21:
                    # PE transpose mode, bf16 PSUM
                    for pair in range(2):
                        psT = psp.tile([128, HB], bf, tag="ps")
                        pv = psT.rearrange("p (c2 fc b) -> p c2 fc b",
                                           c2=2, b=128)
                        for c2 in range(2):
                            i = pair * 2 + c2
                            for fc in range(4):
                                pe.transpose(
                                    pv[:, c2, fc, :],
                                    xb[:, i, 128 * fc : 128 * fc + 128],
                                    ident,
                                )
                        c0 = qh * 4 + pair * 2
                        act.copy(
                            xT[:, :, c0 * 128 : (c0 + 2) * 128].rearrange(
                                "p fc (c2 b) -> p c2 fc b", b=128),
                            pv,
                        )
                    continue
                if stage == 22:
                    # one PSUM bank per c-chunk, ring of 8
                    for i in range(4):
                        c = qh * 4 + i
                        psT = psp.tile([128, 512], f32, tag="pst", bufs=8)
                        pv = psT.rearrange("p (fc b) -> p fc b", b=128)
                        for fc in range(4):
                            pe.matmul(pv[:, fc, :],
                                      lhsT=xb[:, i, 128 * fc : 128 * fc + 128],
                                      rhs=ident, start=True, stop=True)
                        act.copy(xT[:, :, c * 128 : (c + 1) * 128], pv)
                    continue
                for pair in range(2):  # c-pairs within the quarter
                    psT = psp.tile([128, HB], f32, tag="ps")
                    pv = psT.rearrange("p (c2 fc b) -> p c2 fc b", c2=2, b=128)
                    for c2 in range(2):
                        i = pair * 2 + c2
                        for fc in range(4):
                            pe.matmul(
                                pv[:, c2, fc, :],
                                lhsT=xb[:, i, 128 * fc : 128 * fc + 128],
                                rhs=ident,
                                start=True,
                                stop=True,
                            )
                    c0 = qh * 4 + pair * 2
                    if stage == 23:
                        dve.tensor_copy(
                            xT[:, :, c0 * 128 : (c0 + 2) * 128].rearrange(
                                "p fc (c2 b) -> p c2 fc b", b=128),
                            pv,
                        )
                    else:
                        act.copy(
                            xT[:, :, c0 * 128 : (c0 + 2) * 128].rearrange(
                                "p fc (c2 b) -> p c2 fc b", b=128),
                            pv,
                        )

            if stage < 3 or stage in (21, 22, 23):
                nc.sync.dma_start(
                    y[st * ST : (st + 1) * ST, :].rearrange(
                        "(a b) c -> a (b c)", a=1), dummy)
                continue
            if 30 <= stage < 90:
                # TOK perf variants (wrong math, perf probes only)
                for n in range(8):
                    fc = n // 2
                    tok = tkp.tile([128, ST], bf, tag="tok")
                    for h in range(NH):
                        pst = psp.tile([128, HB], f32, tag="ps")
                        for j2 in range(2):
                            js = slice(h * HB + j2 * 512,
                                       h * HB + (j2 + 1) * 512)
                            if stage == 31:  # K=128, offset 0
                                pe.matmul(pst[:, j2 * 512 : (j2 + 1) * 512],
                                          lhsT=wtok[:, fc * 128 : (fc + 1) * 128],
                                          rhs=xT[:, fc, js],
                                          start=True, stop=True)
                            else:  # K=64 offset-0 (32) / as-real (33)
                                k0 = 0 if stage == 32 else (n % 2) * 64
                                pe.matmul(pst[:, j2 * 512 : (j2 + 1) * 512],
                                          lhsT=wtok[k0 : k0 + 64,
                                                    fc * 128 : (fc + 1) * 128],
                                          rhs=xT[k0 : k0 + 64, fc, js],
                                          start=True, stop=True)
                        if stage == 35:
                            dve.tensor_scalar(tok[:, hsl(h)], pst, bcol(n),
                                              0.0, op0=AX.add, op1=AX.max)
                        elif stage == 36:
                            dve.tensor_copy(tok[:, hsl(h)], pst)
                        elif stage == 37:
                            gps.tensor_scalar(tok[:, hsl(h)], pst, bcol(n),
                                              0.0, op0=AX.add, op1=AX.max)
                        else:
                            act.activation(tok[:, hsl(h)], pst, AF.Relu,
                                           bias=bcol(n))
                nc.sync.dma_start(
                    y[st * ST : (st + 1) * ST, :].rearrange(
                        "(a b) c -> a (b c)", a=1), dummy)
                continue
            # ---------- phase TOK: token projections + relu ----------
            toks = []
            for n in range(8):
                fc, half = n // 2, n % 2
                k0 = half * 64
                tok = tkp.tile([128, ST], bf, tag="tok")
                for h in range(NH):
                    pst = psp.tile([128, HB], f32, tag="ps")
                    for j2 in range(2):
                        pe.matmul(
                            pst[:, j2 * 512 : (j2 + 1) * 512],
                            lhsT=wtok[k0 : k0 + 64, fc * 128 : (fc + 1) * 128],
                            rhs=xT[k0 : k0 + 64, fc,
                                   h * HB + j2 * 512 : h * HB + (j2 + 1) * 512],
                            start=True,
                            stop=True,
                        )
                    dve.tensor_scalar(tok[:, hsl(h)], pst, bcol(n), 0.0,
                                      op0=AX.add, op1=AX.max)
                toks.append(tok)

            if stage < 4:
                nc.sync.dma_start(
                    y[st * ST : (st + 1) * ST, :].rearrange(
                        "(a b) c -> a (b c)", a=1), dummy)
                continue
            # ---------- phase ATT ----------
            pq = {}
            for gi, (grp, wof, cof) in enumerate([("A", 0, 14), ("V", 128, 15)]):
                p_sb = mid.tile([128, ST], bf, tag="pq")
                for h in range(NH):
                    pp = psp.tile([128, HB], f32, tag="ps")
                    for j2 in range(2):
                        js = slice(h * HB + j2 * 512, h * HB + (j2 + 1) * 512)
                        pe.matmul(
                            pp[:, j2 * 512 : (j2 + 1) * 512],
                            lhsT=wattn[:, wof : wof + 128],
                            rhs=toks[0][:, js],
                            start=True,
                            stop=True,
                        )
                    act.add(p_sb[:, hsl(h)], pp, bcol(cof))
                pq[grp] = p_sb

            # per m: u = p*tok_m (DVE) -> dot replicated over partitions (PE)
            # -> e_m = exp (ACT) -> fold into running sum + weighted-token acc
            tbars, sums = {}, {}
            for gi, (grp, ms) in enumerate([("A", [1, 2, 3]), ("V", [4, 5, 6, 7])]):
                acc = mid.tile([128, ST], bf, tag="tb")
                tmp = mid.tile([128, ST], bf, tag="tbtmp")
                s_t = mid.tile([128, ST], bf, tag="s")
                prev_e = None
                for mi, m in enumerate(ms):
                    u = up.tile([128, ST], bf, tag="u")
                    dve.tensor_tensor(u, pq[grp], toks[m], op=AX.mult)
                    e_m = ep.tile([128, ST], bf, tag="em")
                    for h in range(NH):
                        pL = psp.tile([128, HB], f32, tag="ps")
                        for j2 in range(2):
                            js = slice(h * HB + j2 * 512,
                                       h * HB + (j2 + 1) * 512)
                            pe.matmul(
                                pL[:, j2 * 512 : (j2 + 1) * 512],
                                lhsT=ones,
                                rhs=u[:, js],
                                start=True,
                                stop=True,
                            )
                        act.activation(e_m[:, hsl(h)], pL, AF.Exp, scale=SCALE)
                    dst = acc if mi == 0 else tmp
                    gps.tensor_tensor(dst, toks[m], e_m, op=AX.mult)
                    if mi > 0:
                        dve.tensor_add(acc, acc, tmp)
                        if mi == 1:
                            dve.tensor_add(s_t, prev_e, e_m)
                        else:
                            gps.tensor_add(s_t, s_t, e_m)
                    prev_e = e_m
                r_t = mid.tile([128, ST], bf, tag="r")
                with nc.allow_low_precision(reason="softmax denom bf16"):
                    dve.reciprocal(r_t, s_t)
                tbars[grp] = acc
                sums[grp] = r_t

            # h = (avW.T tbar) * recip
            hs = {}
            for gi, (grp, wof) in enumerate([("A", 256), ("V", 384)]):
                h_sb = mid.tile([128, ST], bf, tag="hout")
                for h in range(NH):
                    ph = psp.tile([128, HB], f32, tag="ps")
                    for j2 in range(2):
                        js = slice(h * HB + j2 * 512, h * HB + (j2 + 1) * 512)
                        pe.matmul(
                            ph[:, j2 * 512 : (j2 + 1) * 512],
                            lhsT=wattn[:, wof : wof + 128],
                            rhs=tbars[grp][:, js],
                            start=True,
                            stop=True,
                        )
                    dve.tensor_tensor(h_sb[:, hsl(h)], ph,
                                      sums[grp][:, hsl(h)], op=AX.mult)
                hs[grp] = h_sb

            if stage < 5:
                nc.sync.dma_start(
                    y[st * ST : (st + 1) * ST, :].rearrange(
                        "(a b) c -> a (b c)", a=1), dummy)
                continue
            # ---------- gate + mix ----------
            z = one.tile([128, ST], bf, tag="z")
            for h in range(NH):
                pg = psp.tile([128, HB], f32, tag="ps")
                for j2 in range(2):
                    js = slice(h * HB + j2 * 512, h * HB + (j2 + 1) * 512)
                    pe.matmul(pg[:, j2 * 512 : (j2 + 1) * 512],
                              lhsT=wgate[:, 0:128], rhs=hs["A"][:, js],
                              start=True, stop=False)
                    pe.matmul(pg[:, j2 * 512 : (j2 + 1) * 512],
                              lhsT=wgate[:, 128:256], rhs=hs["V"][:, js],
                              start=False, stop=True)
                act.activation(z[:, hsl(h)], pg, AF.Sigmoid, bias=bcol(8))
            dd = one.tile([128, ST], bf, tag="dd")
            dve.tensor_sub(dd, hs["A"], hs["V"])
            zd = up.tile([128, ST], bf, tag="u")
            gps.tensor_tensor(zd, z, dd, op=AX.mult)
            E = dd
            dve.tensor_add(E, zd, hs["V"])

            if stage < 6:
                nc.sync.dma_start(
                    y[st * ST : (st + 1) * ST, :].rearrange(
                        "(a b) c -> a (b c)", a=1), dummy)
                continue
            # ---------- head ----------
            a1 = []
            for mh in range(2):
                t1 = mid.tile([128, ST], bf, tag="a1")
                for h in range(NH):
                    p1 = psp.tile([128, HB], f32, tag="ps")
                    for j2 in range(2):
                        js = slice(h * HB + j2 * 512, h * HB + (j2 + 1) * 512)
                        pe.matmul(p1[:, j2 * 512 : (j2 + 1) * 512],
                                  lhsT=w1sa[:, mh * 128 : (mh + 1) * 128],
                                  rhs=xT[0:64, 0, js], start=True, stop=False)
                        pe.matmul(p1[:, j2 * 512 : (j2 + 1) * 512],
                                  lhsT=w1E[:, mh * 128 : (mh + 1) * 128],
                                  rhs=E[:, js], start=False, stop=True)
                    act.activation(t1[:, hsl(h)], p1, AF.Relu,
                                   bias=bcol(9 + mh))
                a1.append(t1)
            a2 = []
            for mh in range(2):
                t2 = mid.tile([128, ST], bf, tag="a2")
                for h in range(NH):
                    p2 = psp.tile([128, HB], f32, tag="ps")
                    for j2 in range(2):
                        js = slice(h * HB + j2 * 512, h * HB + (j2 + 1) * 512)
                        pe.matmul(p2[:, j2 * 512 : (j2 + 1) * 512],
                                  lhsT=w2[0:128, mh * 128 : (mh + 1) * 128],
                                  rhs=a1[0][:, js], start=True, stop=False)
                        pe.matmul(p2[:, j2 * 512 : (j2 + 1) * 512],
                                  lhsT=w2[0:128,
                                          256 + mh * 128 : 256 + (mh + 1) * 128],
                                  rhs=a1[1][:, js], start=False, stop=True)
                    dve.tensor_scalar(t2[:, hsl(h)], p2, bcol(11 + mh),
                                      0.0, op0=AX.add, op1=AX.max)
                a2.append(t2)
            # drain py directly into natural batch order: half h covers
            # free pos q = c*128+p for c in [8h, 8h+8); batch index p*NBC+c
            yord = one.tile([1, ST], f32, tag="yord", bufs=1)
            yord_v = yord.rearrange("o (p c) -> o p c", c=NBC)
            for h in range(NH):
                py = psp.tile([128, HB], f32, tag="ps")
                for j2 in range(2):
                    js = slice(h * HB + j2 * 512, h * HB + (j2 + 1) * 512)
                    pe.matmul(py[0:1, j2 * 512 : (j2 + 1) * 512],
                              lhsT=w3[:, 0:1], rhs=a2[0][:, js],
                              start=True, stop=False, tile_position=(0, 0))
                    pe.matmul(py[0:1, j2 * 512 : (j2 + 1) * 512],
                              lhsT=w3[:, 1:2], rhs=a2[1][:, js],
                              start=False, stop=True, tile_position=(0, 0))
                act.add(yord_v[:, :, 8 * h : 8 * h + 8],
                        py[0:1, :].rearrange("o (c p) -> o p c", p=128),
                        bm[0:1, 13:14])
            nc.sync.dma_start(
                y[st * ST : (st + 1) * ST, :].rearrange("(a b) c -> a (b c)", a=1),
                yord,
            )

    nc.compile()
    return nc


def _pack_host(inputs):
    f = lambda k: np.asarray(inputs[k], np.float32)
    token_W, token_b = f("token_W"), f("token_b")
    aqW, aqb, akW = f("aqW"), f("aqb"), f("akW")
    avW, avb = f("avW"), f("avb")
    vqW, vqb, vkW = f("vqW"), f("vqb"), f("vkW")
    vvW, vvb = f("vvW"), f("vvb")
    gate_W, gate_b = f("gate_W"), f("gate_b")
    h1W, h1b = f("h1W"), f("h1b")
    h2W, h2b = f("h2W"), f("h2b")
    h3W, h3b = f("h3W"), f("h3b")

    assert np.allclose(avb, vvb), "avb != vvb not supported by fused path"

    wtok = np.zeros((128, 512), np.float32)
    for fc in range(4):
        wtok[0:64, fc * 128 : (fc + 1) * 128] = token_W[2 * fc]
        wtok[64:128, fc * 128 : (fc + 1) * 128] = token_W[2 * fc + 1]

    A_ally = aqW @ akW.T
    A_adv = vqW @ vkW.T
    c_ally = akW @ aqb
    c_adv = vkW @ vqb
    wattn = np.concatenate([A_ally, A_adv, avW, vvW], axis=1)

    gate_b2 = gate_b + gate_W[0:128].T @ avb + gate_W[128:256].T @ vvb
    h1b2 = h1b + h1W[64:192].T @ avb

    wgate = np.concatenate([gate_W[0:128], gate_W[128:256]], axis=1)
    w1sa = h1W[0:64]
    w1E = h1W[64:192]
    w2 = np.concatenate([h2W[0:128], h2W[128:256]], axis=1)
    w3 = np.concatenate([h3W[0:128], h3W[128:256]], axis=1)

    biasm = np.zeros((128, 16), np.float32)
    for n in range(8):
        biasm[:, n] = token_b[n]
    biasm[:, 8] = gate_b2
    biasm[:, 9] = h1b2[0:128]
    biasm[:, 10] = h1b2[128:256]
    biasm[:, 11] = h2b[0:128]
    biasm[:, 12] = h2b[128:256]
    biasm[:, 13] = h3b[0]
    biasm[:, 14] = c_ally
    biasm[:, 15] = c_adv

    shared = {
        "ident": np.eye(128, dtype=BF16),
        "wtok": wtok.astype(BF16),
        "wattn": wattn.astype(BF16),
        "wgate": wgate.astype(BF16),
        "w1sa": w1sa.astype(BF16),
        "w1E": w1E.astype(BF16),
        "w2": w2.astype(BF16),
        "w3": w3.astype(BF16),
        "ones": np.ones((128, 128), BF16),
        "biasm": biasm,
    }
    return shared


_CTX = None


def _build_ctx():
    """Compile the Bass kernel once and wrap it in a sharded jax callable
    with no host-side reshuffling: per-batch inputs are row-sliced by jax
    (views, no copy), weights ride along replicated and device-resident."""
    import jax
    from jax.sharding import Mesh, PartitionSpec, NamedSharding
    from jax.experimental.shard_map import shard_map
    from concourse import bass2jax

    nc = bacc.Bacc("TRN2", target_bir_lowering=False, debug=False,
                   num_devices=1)
    nc = _emit(nc, BPC)
    bass2jax.install_neuronx_cc_hook()

    part_name = nc.partition_id_tensor.name if nc.partition_id_tensor else None
    in_names, out_names, out_avals = [], [], []
    for alloc in nc.m.functions[0].allocations:
        if not isinstance(alloc, mybir.MemoryLocationSet):
            continue
        name = alloc.memorylocations[0].name
        if alloc.kind == "ExternalInput":
            if name != part_name:
                in_names.append(name)
        elif alloc.kind == "ExternalOutput":
            out_names.append(name)
            out_avals.append(jax.core.ShapedArray(
                tuple(alloc.tensor_shape), mybir.dt.np(alloc.dtype)))
    all_names = in_names + out_names + ([part_name] if part_name else [])
    n_in = len(in_names)

    def _body(*args):
        operands = list(args)
        if part_name:
            operands.append(bass2jax.partition_id_tensor())
        return tuple(bass2jax._bass_exec_p.bind(
            *operands,
            out_avals=tuple(out_avals),
            in_names=tuple(all_names),
            out_names=tuple(out_names),
            lowering_input_output_aliases=(),
            sim_require_finite=True,
            sim_require_nnan=True,
            nc=nc,
        ))

    devices = jax.devices()[:N_CORES]
    mesh = Mesh(np.asarray(devices), ("core",))
    batch_names = {"xs", "xa"}
    specs = tuple(
        PartitionSpec("core") if n in batch_names else PartitionSpec()
        for n in in_names
    ) + (PartitionSpec("core"),) * len(out_names)
    fn = jax.jit(
        shard_map(_body, mesh=mesh, in_specs=specs,
                  out_specs=(PartitionSpec("core"),) * len(out_names),
                  check_rep=False),
        keep_unused=True,
    )
    sh_batch = NamedSharding(mesh, PartitionSpec("core"))
    sh_repl = NamedSharding(mesh, PartitionSpec())
    return {
        "jax": jax, "nc": nc, "fn": fn, "in_names": in_names,
        "sh_batch": sh_batch, "sh_repl": sh_repl,
        "y0": jax.device_put(np.zeros((B_FULL, 1), np.float32), sh_batch),
        "wdev": None, "wsrc": None,
    }


def _get_ctx():
    global _CTX
    if _CTX is None:
        _CTX = _build_ctx()
    return _CTX


_WKEYS = ("token_W", "token_b", "aqW", "aqb", "akW", "akb", "avW", "avb",
          "vqW", "vqb", "vkW", "vkb", "vvW", "vvb", "gate_W", "gate_b",
          "h1W", "h1b", "h2W", "h2b", "h3W", "h3b")


def _weights_dev(ctx, inputs):
    src = {k: np.asarray(inputs[k]) for k in _WKEYS}
    if ctx["wsrc"] is not None and all(
        np.array_equal(src[k], ctx["wsrc"][k]) for k in _WKEYS
    ):
        return ctx["wdev"]
    packed = _pack_host(inputs)
    ctx["wdev"] = {
        k: ctx["jax"].device_put(v, ctx["sh_repl"]) for k, v in packed.items()
    }
    ctx["wsrc"] = src
    return ctx["wdev"]


def kernel(**inputs):
    assert int(np.asarray(inputs["current_agent_idx"])) == 0
    states = np.asarray(inputs["states_full"], np.float32)
    actions = np.asarray(inputs["actions_full"], np.float32)
    ctx = _get_ctx()
    wdev = _weights_dev(ctx, inputs)
    dput = ctx["jax"].device_put
    args = []
    for n in ctx["in_names"]:
        if n == "xs":
            args.append(dput(states, ctx["sh_batch"]))
        elif n == "xa":
            args.append(dput(actions, ctx["sh_batch"]))
        else:
            args.append(wdev[n])
    out = ctx["fn"](*args, ctx["y0"])
    return np.asarray(out[0]).astype(np.float32, copy=False)

